# revision 6
# baseline (speedup 1.0000x reference)
"""Trainium2 Bass kernel: CombinedModel = DNN branch (Linear+BatchNorm+ReLU)
+ GCN branch (2x GCNConv -> mean pool) + linear head, on 8 NeuronCores.

v3: batched InstDMAGatherAnt gathers from a bf16 PAIR table + bf16
one-hot matmuls:
- x2 is repacked host-side as x2pair [25000, 128] bf16 (nodes 2i, 2i+1 per
  256B row) so gather indices (src >> 1) fit int16 and rows satisfy the
  256B-multiple element constraint.
- edges sorted by dest block; within a block split into even-src / odd-src
  streams (sorted by src, padded to 128-edge tiles, pad idx 0 / nrm 0); the
  stream parity selects which half of the gathered pair feeds the matmul.
- one dma_gather per (block, stream) on 4 SWDGE queues (~2.4 ns/edge).
- one-hot "stiles" and the aggregation matmuls run in bf16 (fp32 PSUM
  accumulate); self-loops use a no-gather diagonal path from SBUF-resident
  x2 rows of the core's own dest range.
Everything else (DNN branch, algebraic GCN layer-2 collapse, folded head,
single tiny AllReduce) is unchanged from the baseline.
"""

import math
import os
import sys

for _p in ("/opt/trn_rl_repo", "/root/.axon_site/_ro/trn_rl_repo"):
    if os.path.isdir(_p) and _p not in sys.path:
        sys.path.append(_p)

import numpy as np
import ml_dtypes

import concourse.bacc as bacc
import concourse.bass as bass
import concourse.mybir as mybir
import concourse.tile as tile
from concourse import bass_utils
from concourse.masks import make_identity

DT = mybir.dt
ALU = mybir.AluOpType
ACTF = mybir.ActivationFunctionType

N_NODES = 50000
N_EDGES = 800000
BATCH = 16384
DNN_IN = 256
F = 64                       # feature width everywhere in the GNN
CORES = 8
NSH = N_NODES // CORES       # 6250 dest nodes per core
BSH = BATCH // CORES         # 2048 batch rows per core
NBLK = (NSH + 127) // 128    # 49 dest blocks per core
NPAIR = N_NODES // 2         # rows in the bf16 pair table
BN_EPS = 1e-5
GBLK = int(os.environ.get("K_GBLK", "1"))   # dest blocks per gather call pair
NQ = int(os.environ.get("K_NQ", "4"))        # SWDGE queues
GBUFS = int(os.environ.get("K_GBUFS", "6"))  # gather pool bufs (each stream)
QRR = int(os.environ.get("K_QRR", "1"))      # 1 = strict round-robin queues
SCRATCH = int(os.environ.get("K_SCRATCH", "16384"))  # SWDGE ring bytes/queue

_PHASES = dict(dnn=True, gcn=True, head=True, gather=True, stile=True, tail=True)


def _cdiv(a, b):
    return (a + b - 1) // b


def _pack_idx16(idx, tiles):
    """idx [n] -> [128, tiles*8] int16: edge e at (e%16, e//16), replicated
    across the 8 groups of 16 partitions (dma_gather wrap layout)."""
    cols = tiles * 8
    a = np.zeros((16, cols), np.int16)
    n = idx.shape[0]
    assert n <= cols * 16
    a[:, : _cdiv(n, 16)] = (
        np.pad(idx.astype(np.int16), (0, _cdiv(n, 16) * 16 - n))
        .reshape(_cdiv(n, 16), 16).T)
    return np.tile(a, (8, 1))


# --------------------------------------------------------------------------
# Host-side preprocessing: graph indices -> per-core packed gather/one-hot
# metadata with a core-uniform tile structure (SPMD requires one program).
# --------------------------------------------------------------------------

def _prep(inputs):
    x1 = np.asarray(inputs["x1"], np.float32)
    x2 = np.ascontiguousarray(np.asarray(inputs["x2"], np.float32))
    ei = np.asarray(inputs["edge_index"])
    row = ei[0].astype(np.int64)
    col = ei[1].astype(np.int64)

    deg = (np.bincount(col, minlength=N_NODES) + 1.0).astype(np.float32)
    dis = (1.0 / np.sqrt(deg)).astype(np.float32)
    norm = dis[row] * dis[col]

    # layer-2 collapse weights: sum_c out2[c] = sum_n wslf[n] * h2[n] + N*b2
    w_r = np.bincount(row, weights=dis[col].astype(np.float64), minlength=N_NODES)
    wslf = (dis * w_r.astype(np.float32) + dis * dis).astype(np.float32)

    order = np.argsort(col, kind="stable")
    srow = row[order]
    scol = col[order]
    snrm = norm[order]

    # per (core, block, stream) segments; stream 0 = even src, 1 = odd src
    segs = [[None] * NBLK for _ in range(CORES)]
    for k in range(CORES):
        base = k * NSH
        s0 = np.searchsorted(scol, base)
        s1 = np.searchsorted(scol, base + NSH)
        krow = srow[s0:s1]
        knrm = snrm[s0:s1]
        rel = scol[s0:s1] - base
        bst = np.searchsorted(rel, np.arange(NBLK) * 128)
        ben = np.append(bst[1:], rel.size)
        for b in range(NBLK):
            sl = slice(bst[b], ben[b])
            r = krow[sl]
            n = knrm[sl]
            c = (rel[sl] - b * 128).astype(np.float32)
            # sort by src: ascending HBM addresses within each stream
            o = np.argsort(r, kind="stable")
            r, n, c = r[o], n[o], c[o]
            ev = (r % 2) == 0
            segs[k][b] = ((r[ev] >> 1, n[ev], c[ev]),
                          (r[~ev] >> 1, n[~ev], c[~ev]))

    T_LO = [max(_cdiv(segs[k][b][0][0].size, 128) for k in range(CORES))
            for b in range(NBLK)]
    T_HI = [max(_cdiv(segs[k][b][1][0].size, 128) for k in range(CORES))
            for b in range(NBLK)]
    # every block also gets one diag (self-loop) tile, built on-device from
    # the SBUF-resident x2 rows of the core's own dest range
    TLOS, THIS = sum(T_LO), sum(T_HI)

    # gather-call groups of GBLK blocks
    groups = [list(range(g, min(g + GBLK, NBLK)))
              for g in range(0, NBLK, GBLK)]

    def pack_core(k):
        # crl/nrm streams in global tile order:
        #   per group: [lo tiles of its blocks][hi tiles of its blocks]
        # then per block one diag tile appended at the very end (NBLK tiles).
        ntile = TLOS + THIS
        nrm = np.zeros((ntile + NBLK) * 128, np.float32)
        crl = np.zeros((ntile + NBLK) * 128, np.float32)
        idx16_parts = []
        off = 0
        for grp in groups:
            for st in range(2):
                gidx = []
                for b in grp:
                    r, n, c = segs[k][b][st]
                    T = (T_LO if st == 0 else T_HI)[b]
                    m = r.size
                    nrm[off:off + m] = n
                    crl[off:off + m] = c
                    gidx.append(np.pad(r.astype(np.int64), (0, T * 128 - m)))
                    off += T * 128
                gidx = np.concatenate(gidx) if gidx else np.zeros(0, np.int64)
                idx16_parts.append(_pack_idx16(gidx, gidx.size // 128))
        # diag tiles: dest d of block b <- x2[k*NSH + b*128 + d] * dis^2
        for b in range(NBLK):
            nvalid = min(128, NSH - b * 128)
            d2 = (dis[k * NSH + b * 128: k * NSH + b * 128 + nvalid]) ** 2
            nrm[off:off + nvalid] = d2
            crl[off:off + nvalid] = np.arange(nvalid, dtype=np.float32)
            off += 128
        ntot = ntile + NBLK
        nrm_t = np.ascontiguousarray(nrm.reshape(ntot, 128).T)
        crl_t = np.ascontiguousarray(crl.reshape(ntot, 128).T)
        idx16 = np.concatenate(idx16_parts, axis=1)
        wk = np.zeros(NBLK * 128, np.float32)
        wk[:NSH] = wslf[k * NSH:(k + 1) * NSH]
        x2blkT = np.zeros((128, NBLK * F), np.float32)
        xk = x2[k * NSH:(k + 1) * NSH]          # [6250, 64]
        for b in range(NBLK):
            nvalid = min(128, NSH - b * 128)
            x2blkT[:nvalid, b * F:(b + 1) * F] = xk[b * 128: b * 128 + nvalid]
        x2blkT = x2blkT.astype(ml_dtypes.bfloat16)
        return dict(
            idx=idx16, nrm=nrm_t, crl=crl_t,
            wslf=np.ascontiguousarray(wk.reshape(NBLK, 128).T),
            x2blk=x2blkT,
            x1t=None,  # filled below
        )

    per_core = []
    x1t_full = np.ascontiguousarray(x1.T)
    for k in range(CORES):
        m = pack_core(k)
        m["x1t"] = np.ascontiguousarray(x1t_full[:, k * BSH:(k + 1) * BSH])
        per_core.append(m)

    # host-folded head weights (no nonlinearity between fc1 and fc2)
    fc1 = np.asarray(inputs["fc1_W"], np.float32)
    fc2 = np.asarray(inputs["fc2_W"], np.float32)
    u = fc1[F:, :] @ fc2                                    # [64, 1]
    v = np.ascontiguousarray(fc1[:F, :] @ fc2)              # [64, 1]
    z = np.ascontiguousarray(np.asarray(inputs["gcn2_W"], np.float32) @ u)
    c1 = float(np.asarray(inputs["fc1_b"], np.float32) @ fc2[:, 0]
               + np.asarray(inputs["fc2_b"], np.float32)[0]
               + np.asarray(inputs["gcn2_b"], np.float32) @ u[:, 0])

    x2pair = np.ascontiguousarray(
        x2.reshape(NPAIR, 2 * F).astype(ml_dtypes.bfloat16))
    shared = dict(
        x2pair=x2pair,
        w1=np.ascontiguousarray(np.asarray(inputs["gcn1_W"], np.float32)),
        b1b=np.ascontiguousarray(
            np.tile(np.asarray(inputs["gcn1_b"], np.float32), (128, 1))),
        dnnw=np.ascontiguousarray(np.asarray(inputs["dnn_W"], np.float32)),
        gma=np.ascontiguousarray(
            np.asarray(inputs["bn_gamma"], np.float32).reshape(F, 1)),
        bta=np.ascontiguousarray(
            np.asarray(inputs["bn_beta"], np.float32).reshape(F, 1)),
        vc=v, zc=z,
        iota=np.ascontiguousarray(
            np.broadcast_to(np.arange(128, dtype=np.float32), (128, 128))
            .astype(ml_dtypes.bfloat16)),
    )
    return dict(T_LO=tuple(T_LO), T_HI=tuple(T_HI), c1=c1,
                per_core=per_core, shared=shared)


# --------------------------------------------------------------------------
# Device program
# --------------------------------------------------------------------------

def _build_program(T_LO, T_HI, c1, reps=1):
    TLOS, THIS = sum(T_LO), sum(T_HI)
    NTILE = TLOS + THIS + NBLK           # + diag tiles
    groups = [list(range(g, min(g + GBLK, NBLK)))
              for g in range(0, NBLK, GBLK)]

    nc = bacc.Bacc("TRN2", target_bir_lowering=False, debug=False,
                   enable_asserts=False, num_devices=CORES,
                   num_swdge_queues=NQ, dynamic_dma_scratch_size=SCRATCH)
    ap = {}

    def inp(name, shape, dt=DT.float32):
        ap[name] = nc.dram_tensor(name, list(shape), dt,
                                  kind="ExternalInput").ap()

    inp("x2pair", (NPAIR, 2 * F), DT.bfloat16)
    inp("x1t", (DNN_IN, BSH))
    inp("idx", (128, NTILE * 8 - NBLK * 8), DT.int16)
    inp("nrm", (128, NTILE))
    inp("crl", (128, NTILE))
    inp("wslf", (128, NBLK))
    inp("x2blk", (128, NBLK * F), DT.bfloat16)
    inp("w1", (F, F))
    inp("b1b", (128, F))
    inp("dnnw", (DNN_IN, F))
    inp("gma", (F, 1))
    inp("bta", (F, 1))
    inp("vc", (F, 1))
    inp("zc", (F, 1))
    inp("iota", (128, 128), DT.bfloat16)
    out_ap = nc.dram_tensor("out", [1, BSH], DT.float32,
                            kind="ExternalOutput").ap()

    # per-group stream offsets (tiles) and idx16 column offsets
    lo_goff, hi_goff, ic_off = [], [], []
    toff = 0
    icol = 0
    for grp in groups:
        tl = sum(T_LO[b] for b in grp)
        th = sum(T_HI[b] for b in grp)
        lo_goff.append(toff)
        hi_goff.append(toff + tl)
        ic_off.append((icol, icol + tl * 8))
        toff += tl + th
        icol += (tl + th) * 8
    assert toff == TLOS + THIS
    diag_t0 = toff                       # first diag tile index

    with tile.TileContext(nc) as tc:
        with tc.tile_pool(name="const", bufs=1) as cp, \
             tc.tile_pool(name="dram", bufs=1, space="DRAM") as dp:
            def load(name, shape, dt=DT.float32, src=None):
                t = cp.tile(list(shape), dt, tag=name)
                nc.sync.dma_start(out=t[:], in_=src if src is not None
                                  else ap[name][:])
                return t

            w1_sb = load("w1", (F, F))
            b1b_sb = load("b1b", (128, F))
            gma_sb = load("gma", (F, 1))
            bta_sb = load("bta", (F, 1))
            vc_sb = load("vc", (F, 1))
            zc_sb = load("zc", (F, 1))
            iota_sb = load("iota", (128, 128), DT.bfloat16)
            wslf_sb = load("wslf", (128, NBLK))
            x2blk_sb = load("x2blk", (128, NBLK * F), DT.bfloat16)
            dnnw0 = load("dnnw0", (128, F), src=ap["dnnw"][0:128, :])
            dnnw1 = load("dnnw1", (128, F), src=ap["dnnw"][128:256, :])
            x1a = load("x1a", (128, BSH), src=ap["x1t"][0:128, :])
            x1b = load("x1b", (128, BSH), src=ap["x1t"][128:256, :])
            idx_sb = load("idx", (128, NTILE * 8 - NBLK * 8), DT.int16)
            nrm_sb = load("nrm", (128, NTILE))
            crl_sb = load("crl", (128, NTILE))

            ident = cp.tile([128, 128], DT.float32, tag="ident")
            make_identity(nc, ident[:])

            hT = cp.tile([F, BSH], DT.float32, tag="hT")
            sqtmp = cp.tile([F, BSH], DT.float32, tag="sqtmp")
            bn_sum = cp.tile([F, 1], DT.float32, tag="bn_sum")
            bn_sq = cp.tile([F, 1], DT.float32, tag="bn_sq")
            p_acc = cp.tile([F, 1], DT.float32, tag="p_acc")

            def phase_ab():
                if _PHASES["dnn"]:
                    _dnn_phase()
                else:
                    nc.vector.memset(hT[:], 0.0)
                    nc.vector.memset(bn_sum[:], 0.0)
                    nc.vector.memset(bn_sq[:], 1.0)
                    nc.vector.memset(sqtmp[:], 0.0)
                if _PHASES["gcn"]:
                    _gcn_phase()
                else:
                    nc.vector.memset(p_acc[:], 0.0)

            def _dnn_phase():
                with tc.tile_pool(name="pd", bufs=1, space="PSUM") as pd:
                    for c in range(BSH // 512):
                        ps = pd.tile([F, 512], DT.float32)
                        cs = slice(c * 512, (c + 1) * 512)
                        nc.tensor.matmul(out=ps[:], lhsT=dnnw0[:],
                                         rhs=x1a[:, cs], start=True, stop=False)
                        nc.tensor.matmul(out=ps[:], lhsT=dnnw1[:],
                                         rhs=x1b[:, cs], start=False, stop=True)
                        nc.vector.tensor_copy(out=hT[:, cs], in_=ps[:])
                nc.vector.reduce_sum(out=bn_sum[:], in_=hT[:],
                                     axis=mybir.AxisListType.X)
                nc.scalar.activation(out=sqtmp[:], in_=hT[:], func=ACTF.Square,
                                     accum_out=bn_sq[:])

            def _gcn_phase():
                nc.vector.memset(p_acc[:], 0.0)
                # every gather call covers <= CHT tiles (1024 descriptors:
                # the SWDGE ring size; larger calls can wedge the exec unit)
                CHT = 8
                qctr = [0]
                with tc.tile_pool(name="gp", bufs=2 * GBUFS) as gpool, \
                     tc.tile_pool(name="sp", bufs=12) as sp, \
                     tc.tile_pool(name="wp", bufs=4) as wp, \
                     tc.tile_pool(name="pa", bufs=3, space="PSUM") as pa, \
                     tc.tile_pool(name="pt", bufs=1, space="PSUM") as pt, \
                     tc.tile_pool(name="po", bufs=2, space="PSUM") as po, \
                     tc.tile_pool(name="pb", bufs=2, space="PSUM") as pb:

                    def gather_chunks(t0_tile, icol0, ntiles, half):
                        """Gather `ntiles` stream tiles in <=CHT-tile calls.
                        Returns [(pool_tile, local_j, global_g)] per tile."""
                        outs = []
                        for a in range(0, ntiles, CHT):
                            nt = min(CHT, ntiles - a)
                            gt = gpool.tile([128, CHT, 2 * F], DT.bfloat16,
                                            tag="gt")
                            if _PHASES["gather"]:
                                nc.gpsimd.dma_gather(
                                    gt[:, 0:nt, :], ap["x2pair"][:],
                                    idx_sb[:, icol0 + a * 8:
                                           icol0 + (a + nt) * 8],
                                    nt * 128, nt * 128, 2 * F,
                                    single_packet=False,
                                    queue_num=qctr[0] % NQ)
                                qctr[0] += 1
                            else:
                                nc.vector.memset(
                                    gt[:].rearrange("p t e -> p (t e)"), 0.0)
                            for j in range(nt):
                                outs.append((gt, j, t0_tile + a + j, half))
                        return outs

                    for gi, grp in enumerate(groups):
                        tl = sum(T_LO[b] for b in grp)
                        ic0, ic1 = ic_off[gi]
                        lo_base = lo_goff[gi]
                        hi_base = hi_goff[gi]
                        lt_off = 0
                        ht_off = 0
                        for b in grp:
                            ntl, nth = T_LO[b], T_HI[b]
                            tiles = (gather_chunks(lo_base + lt_off,
                                                   ic0 + lt_off * 8, ntl, 0)
                                     + gather_chunks(hi_base + ht_off,
                                                     ic0 + tl * 8 + ht_off * 8,
                                                     nth, 1))
                            agg = pa.tile([128, F], DT.float32)
                            ntot = len(tiles) + 1
                            for ti, (gt, j, g, half) in enumerate(tiles):
                                _edge_mm(sp, agg, iota_sb, crl_sb, nrm_sb, g,
                                         gt[:, j, half * F:(half + 1) * F],
                                         ti, ntot)
                            # diag (self-loop) tile from SBUF-resident x2 rows
                            g = diag_t0 + b
                            _edge_mm(sp, agg, iota_sb, crl_sb, nrm_sb, g,
                                     x2blk_sb[:, b * F:(b + 1) * F],
                                     ntot - 1, ntot)
                            if _PHASES["tail"]:
                                _block_tail(wp, pt, po, pb, agg, b)
                            lt_off += ntl
                            ht_off += nth

            def _edge_mm(sp, agg, iota_sb, crl_sb, nrm_sb, g, rhs, ti, ntot):
                if _PHASES["stile"]:
                    stile = sp.tile([128, 128], DT.bfloat16, tag="stile")
                    nc.vector.tensor_scalar(
                        out=stile[:], in0=iota_sb[:],
                        scalar1=crl_sb[:, g:g + 1],
                        scalar2=nrm_sb[:, g:g + 1],
                        op0=ALU.is_equal, op1=ALU.mult)
                    lhsT = stile[:]
                else:
                    lhsT = iota_sb[:]
                nc.tensor.matmul(out=agg[:], lhsT=lhsT, rhs=rhs,
                                 start=(ti == 0), stop=(ti == ntot - 1))

            def _block_tail(wp, pt, po, pb, agg, b):
                aggsb = wp.tile([128, F], DT.float32, tag="aggsb")
                nc.scalar.activation(out=aggsb[:], in_=agg[:], func=ACTF.Copy)
                pst = pt.tile([F, 128], DT.float32)
                nc.tensor.transpose(out=pst[:], in_=aggsb[:],
                                    identity=ident[:])
                aggT = wp.tile([F, 128], DT.float32, tag="aggT")
                nc.scalar.activation(out=aggT[:], in_=pst[:], func=ACTF.Copy)
                o1 = po.tile([128, F], DT.float32)
                nc.tensor.matmul(out=o1[:], lhsT=aggT[:], rhs=w1_sb[:],
                                 start=True, stop=True)
                g1 = wp.tile([128, F], DT.float32, tag="g1")
                nc.vector.tensor_tensor(out=g1[:], in0=o1[:], in1=b1b_sb[:],
                                        op=ALU.add)
                nc.scalar.activation(out=g1[:], in_=g1[:], func=ACTF.Relu)
                pbt = pb.tile([F, 1], DT.float32)
                nc.tensor.matmul(out=pbt[:], lhsT=g1[:],
                                 rhs=wslf_sb[:, b:b + 1], start=True,
                                 stop=True)
                nc.vector.tensor_tensor(out=p_acc[:], in0=p_acc[:],
                                        in1=pbt[:], op=ALU.add)

            if reps == 1:
                phase_ab()
            else:
                with tc.For_i(0, reps, 1):
                    phase_ab()

            # ---------------- cross-core stats + head ----------------
            stats = cp.tile([F, 4], DT.float32, tag="stats")
            nc.vector.tensor_copy(out=stats[:, 0:1], in_=bn_sum[:])
            nc.vector.tensor_copy(out=stats[:, 1:2], in_=bn_sq[:])
            nc.vector.tensor_copy(out=stats[:, 2:3], in_=p_acc[:])
            cc_in = dp.tile([F, 3], DT.float32)
            cc_out = dp.tile([F, 3], DT.float32)
            nc.gpsimd.dma_start(out=cc_in[:], in_=stats[:, 0:3])
            nc.gpsimd.collective_compute(
                "AllReduce", ALU.add,
                replica_groups=[list(range(CORES))],
                ins=[cc_in.opt()], outs=[cc_out.opt()],
            )
            tot = cp.tile([F, 3], DT.float32, tag="tot")
            nc.sync.dma_start(out=tot[:], in_=cc_out[:])

            if not _PHASES["head"]:
                outsb0 = cp.tile([1, BSH], DT.float32, tag="outsb")
                nc.vector.memset(outsb0[:], 0.0)
                nc.sync.dma_start(out=out_ap[:], in_=outsb0[:])
            if _PHASES["head"]:
              with tc.tile_pool(name="pc", bufs=2, space="PSUM") as pc:
                sm = cp
                mu = sm.tile([F, 1], DT.float32, tag="mu")
                nc.vector.tensor_scalar(out=mu[:], in0=tot[:, 0:1],
                                        scalar1=1.0 / BATCH, scalar2=None,
                                        op0=ALU.mult)
                ex2 = sm.tile([F, 1], DT.float32, tag="ex2")
                nc.vector.tensor_scalar(out=ex2[:], in0=tot[:, 1:2],
                                        scalar1=1.0 / BATCH, scalar2=None,
                                        op0=ALU.mult)
                m2 = sm.tile([F, 1], DT.float32, tag="m2")
                nc.vector.tensor_tensor(out=m2[:], in0=mu[:], in1=mu[:],
                                        op=ALU.mult)
                var = sm.tile([F, 1], DT.float32, tag="var")
                nc.vector.tensor_tensor(out=var[:], in0=ex2[:], in1=m2[:],
                                        op=ALU.subtract)
                vp = sm.tile([F, 1], DT.float32, tag="vp")
                nc.vector.tensor_scalar(out=vp[:], in0=var[:],
                                        scalar1=BN_EPS, scalar2=None,
                                        op0=ALU.add)
                sd = sm.tile([F, 1], DT.float32, tag="sd")
                nc.scalar.activation(out=sd[:], in_=vp[:], func=ACTF.Sqrt)
                istd = sm.tile([F, 1], DT.float32, tag="istd")
                nc.vector.reciprocal(out=istd[:], in_=sd[:])
                scl = sm.tile([F, 1], DT.float32, tag="scl")
                nc.vector.tensor_tensor(out=scl[:], in0=istd[:], in1=gma_sb[:],
                                        op=ALU.mult)
                msc = sm.tile([F, 1], DT.float32, tag="msc")
                nc.vector.tensor_tensor(out=msc[:], in0=mu[:], in1=scl[:],
                                        op=ALU.mult)
                shf = sm.tile([F, 1], DT.float32, tag="shf")
                nc.vector.tensor_tensor(out=shf[:], in0=bta_sb[:], in1=msc[:],
                                        op=ALU.subtract)
                nc.scalar.activation(out=hT[:], in_=hT[:], func=ACTF.Relu,
                                     scale=scl[:, :], bias=shf[:, :])
                s0p = pc.tile([1, 1], DT.float32, tag="s0p")
                nc.tensor.matmul(out=s0p[:], lhsT=zc_sb[:], rhs=tot[:, 2:3],
                                 start=True, stop=True)
                s0 = sm.tile([1, 1], DT.float32, tag="s0")
                nc.vector.tensor_scalar(out=s0[:], in0=s0p[:],
                                        scalar1=1.0 / N_NODES, scalar2=c1,
                                        op0=ALU.mult, op1=ALU.add)
                outsb = cp.tile([1, BSH], DT.float32, tag="outsb")
                for c in range(BSH // 512):
                    cs = slice(c * 512, (c + 1) * 512)
                    pov = pc.tile([1, 512], DT.float32, tag="pov")
                    nc.tensor.matmul(out=pov[:], lhsT=vc_sb[:], rhs=hT[:, cs],
                                     start=True, stop=True)
                    nc.vector.tensor_scalar(out=outsb[:, cs], in0=pov[:],
                                            scalar1=s0[:, :], scalar2=None,
                                            op0=ALU.add)
                nc.sync.dma_start(out=out_ap[:], in_=outsb[:])

    nc.compile()
    return nc


_CACHE = {}


def _get_program(T_LO, T_HI, c1, reps=1):
    key = (tuple(T_LO), tuple(T_HI), float(c1), reps)
    if key not in _CACHE:
        _CACHE[key] = _build_program(tuple(T_LO), tuple(T_HI), c1, reps)
    return _CACHE[key]


def _in_maps(st):
    maps = []
    for k in range(CORES):
        m = dict(st["shared"])
        m.update(st["per_core"][k])
        maps.append(m)
    return maps


def kernel(**inputs):
    st = _prep(inputs)
    nc = _get_program(st["T_LO"], st["T_HI"], st["c1"], reps=1)
    res = bass_utils.run_bass_kernel_spmd(
        nc, _in_maps(st), core_ids=list(range(CORES)))
    out = np.concatenate(
        [res.results[k]["out"].reshape(BSH, 1) for k in range(CORES)], axis=0)
    return out.astype(np.float32)


# revision 8
# speedup vs baseline: 1.3788x; 1.3788x over previous
"""Trainium2 Bass kernel: CombinedModel = DNN branch (Linear+BatchNorm+ReLU)
+ GCN branch (2x GCNConv -> mean pool) + linear head, on 8 NeuronCores.

v3: batched InstDMAGatherAnt gathers from a bf16 PAIR table + bf16
one-hot matmuls:
- x2 is repacked host-side as x2pair [25000, 128] bf16 (nodes 2i, 2i+1 per
  256B row) so gather indices (src >> 1) fit int16 and rows satisfy the
  256B-multiple element constraint.
- edges sorted by dest block; within a block split into even-src / odd-src
  streams (sorted by src, padded to 128-edge tiles, pad idx 0 / nrm 0); the
  stream parity selects which half of the gathered pair feeds the matmul.
- dma_gather calls capped at 8 tiles (1024 descriptors = one SWDGE ring)
  round-robin across 4 SWDGE queues (~2.4-6 ns/edge); larger calls overflow
  the ring and can wedge the exec unit.
- one-hot "stiles" and the aggregation matmuls run in bf16 (fp32 PSUM
  accumulate); self-loops use a no-gather diagonal path from SBUF-resident
  x2 rows of the core's own dest range.
Everything else (DNN branch, algebraic GCN layer-2 collapse, folded head,
single tiny AllReduce) is unchanged from the baseline.
"""

import math
import os
import sys

for _p in ("/opt/trn_rl_repo", "/root/.axon_site/_ro/trn_rl_repo"):
    if os.path.isdir(_p) and _p not in sys.path:
        sys.path.append(_p)

import numpy as np
import ml_dtypes

import concourse.bacc as bacc
import concourse.bass as bass
import concourse.mybir as mybir
import concourse.tile as tile
from concourse import bass_utils
from concourse.masks import make_identity

DT = mybir.dt
ALU = mybir.AluOpType
ACTF = mybir.ActivationFunctionType

N_NODES = 50000
N_EDGES = 800000
BATCH = 16384
DNN_IN = 256
F = 64                       # feature width everywhere in the GNN
CORES = 8
NSH = N_NODES // CORES       # 6250 dest nodes per core
BSH = BATCH // CORES         # 2048 batch rows per core
NBLK = (NSH + 127) // 128    # 49 dest blocks per core
NPAIR = N_NODES // 2         # rows in the bf16 pair table
BN_EPS = 1e-5
GBLK = int(os.environ.get("K_GBLK", "1"))   # dest blocks per gather call pair
NQ = int(os.environ.get("K_NQ", "4"))        # SWDGE queues
GBUFS = int(os.environ.get("K_GBUFS", "6"))  # gather pool bufs (each stream)
QRR = int(os.environ.get("K_QRR", "1"))      # 1 = strict round-robin queues
SCRATCH = int(os.environ.get("K_SCRATCH", "32768"))  # SWDGE ring bytes/queue

_PHASES = dict(dnn=True, gcn=True, head=True, gather=True, stile=True, tail=True)


def _cdiv(a, b):
    return (a + b - 1) // b


def _pack_idx16(idx, tiles):
    """idx [n] -> [128, tiles*8] int16: edge e at (e%16, e//16), replicated
    across the 8 groups of 16 partitions (dma_gather wrap layout)."""
    cols = tiles * 8
    a = np.zeros((16, cols), np.int16)
    n = idx.shape[0]
    assert n <= cols * 16
    a[:, : _cdiv(n, 16)] = (
        np.pad(idx.astype(np.int16), (0, _cdiv(n, 16) * 16 - n))
        .reshape(_cdiv(n, 16), 16).T)
    return np.tile(a, (8, 1))


# --------------------------------------------------------------------------
# Host-side preprocessing: graph indices -> per-core packed gather/one-hot
# metadata with a core-uniform tile structure (SPMD requires one program).
# --------------------------------------------------------------------------

def _prep(inputs):
    x1 = np.asarray(inputs["x1"], np.float32)
    x2 = np.ascontiguousarray(np.asarray(inputs["x2"], np.float32))
    ei = np.asarray(inputs["edge_index"])
    row = ei[0].astype(np.int64)
    col = ei[1].astype(np.int64)

    deg = (np.bincount(col, minlength=N_NODES) + 1.0).astype(np.float32)
    dis = (1.0 / np.sqrt(deg)).astype(np.float32)
    norm = dis[row] * dis[col]

    # layer-2 collapse weights: sum_c out2[c] = sum_n wslf[n] * h2[n] + N*b2
    w_r = np.bincount(row, weights=dis[col].astype(np.float64), minlength=N_NODES)
    wslf = (dis * w_r.astype(np.float32) + dis * dis).astype(np.float32)

    order = np.argsort(col, kind="stable")
    srow = row[order]
    scol = col[order]
    snrm = norm[order]

    # per (core, block, stream) segments; stream 0 = even src, 1 = odd src
    segs = [[None] * NBLK for _ in range(CORES)]
    for k in range(CORES):
        base = k * NSH
        s0 = np.searchsorted(scol, base)
        s1 = np.searchsorted(scol, base + NSH)
        krow = srow[s0:s1]
        knrm = snrm[s0:s1]
        rel = scol[s0:s1] - base
        bst = np.searchsorted(rel, np.arange(NBLK) * 128)
        ben = np.append(bst[1:], rel.size)
        for b in range(NBLK):
            sl = slice(bst[b], ben[b])
            r = krow[sl]
            n = knrm[sl]
            c = (rel[sl] - b * 128).astype(np.float32)
            # sort by src: ascending HBM addresses within each stream
            o = np.argsort(r, kind="stable")
            r, n, c = r[o], n[o], c[o]
            ev = (r % 2) == 0
            segs[k][b] = ((r[ev] >> 1, n[ev], c[ev]),
                          (r[~ev] >> 1, n[~ev], c[~ev]))

    T_LO = [max(_cdiv(segs[k][b][0][0].size, 128) for k in range(CORES))
            for b in range(NBLK)]
    T_HI = [max(_cdiv(segs[k][b][1][0].size, 128) for k in range(CORES))
            for b in range(NBLK)]
    # every block also gets one diag (self-loop) tile, built on-device from
    # the SBUF-resident x2 rows of the core's own dest range
    TLOS, THIS = sum(T_LO), sum(T_HI)

    # gather-call groups of GBLK blocks
    groups = [list(range(g, min(g + GBLK, NBLK)))
              for g in range(0, NBLK, GBLK)]

    def pack_core(k):
        # crl/nrm streams in global tile order:
        #   per group: [lo tiles of its blocks][hi tiles of its blocks]
        # then per block one diag tile appended at the very end (NBLK tiles).
        ntile = TLOS + THIS
        nrm = np.zeros((ntile + NBLK) * 128, np.float32)
        crl = np.zeros((ntile + NBLK) * 128, np.float32)
        idx16_parts = []
        off = 0
        for grp in groups:
            for st in range(2):
                gidx = []
                for b in grp:
                    r, n, c = segs[k][b][st]
                    T = (T_LO if st == 0 else T_HI)[b]
                    m = r.size
                    nrm[off:off + m] = n
                    crl[off:off + m] = c
                    gidx.append(np.pad(r.astype(np.int64), (0, T * 128 - m)))
                    off += T * 128
                gidx = np.concatenate(gidx) if gidx else np.zeros(0, np.int64)
                idx16_parts.append(_pack_idx16(gidx, gidx.size // 128))
        # diag tiles: dest d of block b <- x2[k*NSH + b*128 + d] * dis^2
        for b in range(NBLK):
            nvalid = min(128, NSH - b * 128)
            d2 = (dis[k * NSH + b * 128: k * NSH + b * 128 + nvalid]) ** 2
            nrm[off:off + nvalid] = d2
            crl[off:off + nvalid] = np.arange(nvalid, dtype=np.float32)
            off += 128
        ntot = ntile + NBLK
        nrm_t = np.ascontiguousarray(nrm.reshape(ntot, 128).T)
        crl_t = np.ascontiguousarray(crl.reshape(ntot, 128).T)
        idx16 = np.concatenate(idx16_parts, axis=1)
        wk = np.zeros(NBLK * 128, np.float32)
        wk[:NSH] = wslf[k * NSH:(k + 1) * NSH]
        x2blkT = np.zeros((128, NBLK * F), np.float32)
        xk = x2[k * NSH:(k + 1) * NSH]          # [6250, 64]
        for b in range(NBLK):
            nvalid = min(128, NSH - b * 128)
            x2blkT[:nvalid, b * F:(b + 1) * F] = xk[b * 128: b * 128 + nvalid]
        x2blkT = x2blkT.astype(ml_dtypes.bfloat16)
        return dict(
            idx=idx16, nrm=nrm_t, crl=crl_t,
            wslf=np.ascontiguousarray(wk.reshape(NBLK, 128).T),
            x2blk=x2blkT,
            x1t=None,  # filled below
        )

    per_core = []
    x1t_full = np.ascontiguousarray(x1.T)
    for k in range(CORES):
        m = pack_core(k)
        m["x1t"] = np.ascontiguousarray(x1t_full[:, k * BSH:(k + 1) * BSH])
        per_core.append(m)

    # host-folded head weights (no nonlinearity between fc1 and fc2)
    fc1 = np.asarray(inputs["fc1_W"], np.float32)
    fc2 = np.asarray(inputs["fc2_W"], np.float32)
    u = fc1[F:, :] @ fc2                                    # [64, 1]
    v = np.ascontiguousarray(fc1[:F, :] @ fc2)              # [64, 1]
    z = np.ascontiguousarray(np.asarray(inputs["gcn2_W"], np.float32) @ u)
    c1 = float(np.asarray(inputs["fc1_b"], np.float32) @ fc2[:, 0]
               + np.asarray(inputs["fc2_b"], np.float32)[0]
               + np.asarray(inputs["gcn2_b"], np.float32) @ u[:, 0])

    x2pair = np.ascontiguousarray(
        x2.reshape(NPAIR, 2 * F).astype(ml_dtypes.bfloat16))
    shared = dict(
        x2pair=x2pair,
        w1=np.ascontiguousarray(np.asarray(inputs["gcn1_W"], np.float32)),
        b1b=np.ascontiguousarray(
            np.tile(np.asarray(inputs["gcn1_b"], np.float32), (128, 1))),
        dnnw=np.ascontiguousarray(np.asarray(inputs["dnn_W"], np.float32)),
        gma=np.ascontiguousarray(
            np.asarray(inputs["bn_gamma"], np.float32).reshape(F, 1)),
        bta=np.ascontiguousarray(
            np.asarray(inputs["bn_beta"], np.float32).reshape(F, 1)),
        vc=v, zc=z,
        iota=np.ascontiguousarray(
            np.broadcast_to(np.arange(128, dtype=np.float32), (128, 128))
            .astype(ml_dtypes.bfloat16)),
    )
    return dict(T_LO=tuple(T_LO), T_HI=tuple(T_HI), c1=c1,
                per_core=per_core, shared=shared)


# --------------------------------------------------------------------------
# Device program
# --------------------------------------------------------------------------

def _build_program(T_LO, T_HI, c1, reps=1):
    TLOS, THIS = sum(T_LO), sum(T_HI)
    NTILE = TLOS + THIS + NBLK           # + diag tiles
    groups = [list(range(g, min(g + GBLK, NBLK)))
              for g in range(0, NBLK, GBLK)]

    nc = bacc.Bacc("TRN2", target_bir_lowering=False, debug=False,
                   enable_asserts=False, num_devices=CORES,
                   num_swdge_queues=NQ, dynamic_dma_scratch_size=SCRATCH)
    ap = {}

    def inp(name, shape, dt=DT.float32):
        ap[name] = nc.dram_tensor(name, list(shape), dt,
                                  kind="ExternalInput").ap()

    inp("x2pair", (NPAIR, 2 * F), DT.bfloat16)
    inp("x1t", (DNN_IN, BSH))
    inp("idx", (128, NTILE * 8 - NBLK * 8), DT.int16)
    inp("nrm", (128, NTILE))
    inp("crl", (128, NTILE))
    inp("wslf", (128, NBLK))
    inp("x2blk", (128, NBLK * F), DT.bfloat16)
    inp("w1", (F, F))
    inp("b1b", (128, F))
    inp("dnnw", (DNN_IN, F))
    inp("gma", (F, 1))
    inp("bta", (F, 1))
    inp("vc", (F, 1))
    inp("zc", (F, 1))
    inp("iota", (128, 128), DT.bfloat16)
    out_ap = nc.dram_tensor("out", [1, BSH], DT.float32,
                            kind="ExternalOutput").ap()

    # per-group stream offsets (tiles) and idx16 column offsets
    lo_goff, hi_goff, ic_off = [], [], []
    toff = 0
    icol = 0
    for grp in groups:
        tl = sum(T_LO[b] for b in grp)
        th = sum(T_HI[b] for b in grp)
        lo_goff.append(toff)
        hi_goff.append(toff + tl)
        ic_off.append((icol, icol + tl * 8))
        toff += tl + th
        icol += (tl + th) * 8
    assert toff == TLOS + THIS
    diag_t0 = toff                       # first diag tile index

    with tile.TileContext(nc) as tc:
        with tc.tile_pool(name="const", bufs=1) as cp, \
             tc.tile_pool(name="dram", bufs=1, space="DRAM") as dp:
            def load(name, shape, dt=DT.float32, src=None):
                t = cp.tile(list(shape), dt, tag=name)
                nc.sync.dma_start(out=t[:], in_=src if src is not None
                                  else ap[name][:])
                return t

            w1_sb = load("w1", (F, F))
            b1b_sb = load("b1b", (128, F))
            gma_sb = load("gma", (F, 1))
            bta_sb = load("bta", (F, 1))
            vc_sb = load("vc", (F, 1))
            zc_sb = load("zc", (F, 1))
            iota_sb = load("iota", (128, 128), DT.bfloat16)
            wslf_sb = load("wslf", (128, NBLK))
            x2blk_sb = load("x2blk", (128, NBLK * F), DT.bfloat16)
            dnnw0 = load("dnnw0", (128, F), src=ap["dnnw"][0:128, :])
            dnnw1 = load("dnnw1", (128, F), src=ap["dnnw"][128:256, :])
            x1a = load("x1a", (128, BSH), src=ap["x1t"][0:128, :])
            x1b = load("x1b", (128, BSH), src=ap["x1t"][128:256, :])
            idx_sb = load("idx", (128, NTILE * 8 - NBLK * 8), DT.int16)
            nrm_sb = load("nrm", (128, NTILE))
            crl_sb = load("crl", (128, NTILE))

            ident = cp.tile([128, 128], DT.float32, tag="ident")
            make_identity(nc, ident[:])

            hT = cp.tile([F, BSH], DT.float32, tag="hT")
            sqtmp = cp.tile([F, BSH], DT.float32, tag="sqtmp")
            bn_sum = cp.tile([F, 1], DT.float32, tag="bn_sum")
            bn_sq = cp.tile([F, 1], DT.float32, tag="bn_sq")
            p_acc = cp.tile([F, 1], DT.float32, tag="p_acc")

            def phase_ab():
                if _PHASES["dnn"]:
                    _dnn_phase()
                else:
                    nc.vector.memset(hT[:], 0.0)
                    nc.vector.memset(bn_sum[:], 0.0)
                    nc.vector.memset(bn_sq[:], 1.0)
                    nc.vector.memset(sqtmp[:], 0.0)
                if _PHASES["gcn"]:
                    _gcn_phase()
                else:
                    nc.vector.memset(p_acc[:], 0.0)

            def _dnn_phase():
                with tc.tile_pool(name="pd", bufs=1, space="PSUM") as pd:
                    for c in range(BSH // 512):
                        ps = pd.tile([F, 512], DT.float32)
                        cs = slice(c * 512, (c + 1) * 512)
                        nc.tensor.matmul(out=ps[:], lhsT=dnnw0[:],
                                         rhs=x1a[:, cs], start=True, stop=False)
                        nc.tensor.matmul(out=ps[:], lhsT=dnnw1[:],
                                         rhs=x1b[:, cs], start=False, stop=True)
                        nc.vector.tensor_copy(out=hT[:, cs], in_=ps[:])
                nc.vector.reduce_sum(out=bn_sum[:], in_=hT[:],
                                     axis=mybir.AxisListType.X)
                nc.scalar.activation(out=sqtmp[:], in_=hT[:], func=ACTF.Square,
                                     accum_out=bn_sq[:])

            def _gcn_phase():
                nc.vector.memset(p_acc[:], 0.0)
                # every gather call covers <= CHT tiles (1024 descriptors:
                # the SWDGE ring size; larger calls can wedge the exec unit)
                CHT = 8
                qctr = [0]
                with tc.tile_pool(name="gp", bufs=2 * GBUFS) as gpool, \
                     tc.tile_pool(name="sp", bufs=12) as sp, \
                     tc.tile_pool(name="wp", bufs=4) as wp, \
                     tc.tile_pool(name="pa", bufs=3, space="PSUM") as pa, \
                     tc.tile_pool(name="pt", bufs=1, space="PSUM") as pt, \
                     tc.tile_pool(name="po", bufs=2, space="PSUM") as po, \
                     tc.tile_pool(name="pb", bufs=2, space="PSUM") as pb:

                    def gather_chunks(t0_tile, icol0, ntiles, half):
                        """Gather `ntiles` stream tiles in <=CHT-tile calls
                        of balanced sizes (11 -> 6+5, not 8+3).
                        Returns [(pool_tile, local_j, global_g)] per tile."""
                        outs = []
                        nch = _cdiv(ntiles, CHT)
                        a = 0
                        for ci in range(nch):
                            nt = _cdiv(ntiles - a, nch - ci)
                            gt = gpool.tile([128, CHT, 2 * F], DT.bfloat16,
                                            tag="gt")
                            if _PHASES["gather"]:
                                nc.gpsimd.dma_gather(
                                    gt[:, 0:nt, :], ap["x2pair"][:],
                                    idx_sb[:, icol0 + a * 8:
                                           icol0 + (a + nt) * 8],
                                    nt * 128, nt * 128, 2 * F,
                                    single_packet=False,
                                    queue_num=qctr[0] % NQ)
                                qctr[0] += 1
                            else:
                                nc.vector.memset(
                                    gt[:].rearrange("p t e -> p (t e)"), 0.0)
                            for j in range(nt):
                                outs.append((gt, j, t0_tile + a + j, half))
                            a += nt
                        return outs

                    for gi, grp in enumerate(groups):
                        tl = sum(T_LO[b] for b in grp)
                        ic0, ic1 = ic_off[gi]
                        lo_base = lo_goff[gi]
                        hi_base = hi_goff[gi]
                        lt_off = 0
                        ht_off = 0
                        for b in grp:
                            ntl, nth = T_LO[b], T_HI[b]
                            tiles = (gather_chunks(lo_base + lt_off,
                                                   ic0 + lt_off * 8, ntl, 0)
                                     + gather_chunks(hi_base + ht_off,
                                                     ic0 + tl * 8 + ht_off * 8,
                                                     nth, 1))
                            agg = pa.tile([128, F], DT.float32)
                            ntot = len(tiles) + 1
                            for ti, (gt, j, g, half) in enumerate(tiles):
                                _edge_mm(sp, agg, iota_sb, crl_sb, nrm_sb, g,
                                         gt[:, j, half * F:(half + 1) * F],
                                         ti, ntot)
                            # diag (self-loop) tile from SBUF-resident x2 rows
                            g = diag_t0 + b
                            _edge_mm(sp, agg, iota_sb, crl_sb, nrm_sb, g,
                                     x2blk_sb[:, b * F:(b + 1) * F],
                                     ntot - 1, ntot)
                            if _PHASES["tail"]:
                                _block_tail(wp, pt, po, pb, agg, b)
                            lt_off += ntl
                            ht_off += nth

            def _edge_mm(sp, agg, iota_sb, crl_sb, nrm_sb, g, rhs, ti, ntot):
                if _PHASES["stile"]:
                    stile = sp.tile([128, 128], DT.bfloat16, tag="stile")
                    nc.vector.tensor_scalar(
                        out=stile[:], in0=iota_sb[:],
                        scalar1=crl_sb[:, g:g + 1],
                        scalar2=nrm_sb[:, g:g + 1],
                        op0=ALU.is_equal, op1=ALU.mult)
                    lhsT = stile[:]
                else:
                    lhsT = iota_sb[:]
                nc.tensor.matmul(out=agg[:], lhsT=lhsT, rhs=rhs,
                                 start=(ti == 0), stop=(ti == ntot - 1))

            def _block_tail(wp, pt, po, pb, agg, b):
                aggsb = wp.tile([128, F], DT.float32, tag="aggsb")
                nc.scalar.activation(out=aggsb[:], in_=agg[:], func=ACTF.Copy)
                pst = pt.tile([F, 128], DT.float32)
                nc.tensor.transpose(out=pst[:], in_=aggsb[:],
                                    identity=ident[:])
                aggT = wp.tile([F, 128], DT.float32, tag="aggT")
                nc.scalar.activation(out=aggT[:], in_=pst[:], func=ACTF.Copy)
                o1 = po.tile([128, F], DT.float32)
                nc.tensor.matmul(out=o1[:], lhsT=aggT[:], rhs=w1_sb[:],
                                 start=True, stop=True)
                g1 = wp.tile([128, F], DT.float32, tag="g1")
                nc.vector.tensor_tensor(out=g1[:], in0=o1[:], in1=b1b_sb[:],
                                        op=ALU.add)
                nc.scalar.activation(out=g1[:], in_=g1[:], func=ACTF.Relu)
                pbt = pb.tile([F, 1], DT.float32)
                nc.tensor.matmul(out=pbt[:], lhsT=g1[:],
                                 rhs=wslf_sb[:, b:b + 1], start=True,
                                 stop=True)
                nc.vector.tensor_tensor(out=p_acc[:], in0=p_acc[:],
                                        in1=pbt[:], op=ALU.add)

            if reps == 1:
                phase_ab()
            else:
                with tc.For_i(0, reps, 1):
                    phase_ab()

            # ---------------- cross-core stats + head ----------------
            stats = cp.tile([F, 4], DT.float32, tag="stats")
            nc.vector.tensor_copy(out=stats[:, 0:1], in_=bn_sum[:])
            nc.vector.tensor_copy(out=stats[:, 1:2], in_=bn_sq[:])
            nc.vector.tensor_copy(out=stats[:, 2:3], in_=p_acc[:])
            cc_in = dp.tile([F, 3], DT.float32)
            cc_out = dp.tile([F, 3], DT.float32)
            nc.gpsimd.dma_start(out=cc_in[:], in_=stats[:, 0:3])
            nc.gpsimd.collective_compute(
                "AllReduce", ALU.add,
                replica_groups=[list(range(CORES))],
                ins=[cc_in.opt()], outs=[cc_out.opt()],
            )
            tot = cp.tile([F, 3], DT.float32, tag="tot")
            nc.sync.dma_start(out=tot[:], in_=cc_out[:])

            if not _PHASES["head"]:
                outsb0 = cp.tile([1, BSH], DT.float32, tag="outsb")
                nc.vector.memset(outsb0[:], 0.0)
                nc.sync.dma_start(out=out_ap[:], in_=outsb0[:])
            if _PHASES["head"]:
              with tc.tile_pool(name="pc", bufs=2, space="PSUM") as pc:
                sm = cp
                mu = sm.tile([F, 1], DT.float32, tag="mu")
                nc.vector.tensor_scalar(out=mu[:], in0=tot[:, 0:1],
                                        scalar1=1.0 / BATCH, scalar2=None,
                                        op0=ALU.mult)
                ex2 = sm.tile([F, 1], DT.float32, tag="ex2")
                nc.vector.tensor_scalar(out=ex2[:], in0=tot[:, 1:2],
                                        scalar1=1.0 / BATCH, scalar2=None,
                                        op0=ALU.mult)
                m2 = sm.tile([F, 1], DT.float32, tag="m2")
                nc.vector.tensor_tensor(out=m2[:], in0=mu[:], in1=mu[:],
                                        op=ALU.mult)
                var = sm.tile([F, 1], DT.float32, tag="var")
                nc.vector.tensor_tensor(out=var[:], in0=ex2[:], in1=m2[:],
                                        op=ALU.subtract)
                vp = sm.tile([F, 1], DT.float32, tag="vp")
                nc.vector.tensor_scalar(out=vp[:], in0=var[:],
                                        scalar1=BN_EPS, scalar2=None,
                                        op0=ALU.add)
                sd = sm.tile([F, 1], DT.float32, tag="sd")
                nc.scalar.activation(out=sd[:], in_=vp[:], func=ACTF.Sqrt)
                istd = sm.tile([F, 1], DT.float32, tag="istd")
                nc.vector.reciprocal(out=istd[:], in_=sd[:])
                scl = sm.tile([F, 1], DT.float32, tag="scl")
                nc.vector.tensor_tensor(out=scl[:], in0=istd[:], in1=gma_sb[:],
                                        op=ALU.mult)
                msc = sm.tile([F, 1], DT.float32, tag="msc")
                nc.vector.tensor_tensor(out=msc[:], in0=mu[:], in1=scl[:],
                                        op=ALU.mult)
                shf = sm.tile([F, 1], DT.float32, tag="shf")
                nc.vector.tensor_tensor(out=shf[:], in0=bta_sb[:], in1=msc[:],
                                        op=ALU.subtract)
                nc.scalar.activation(out=hT[:], in_=hT[:], func=ACTF.Relu,
                                     scale=scl[:, :], bias=shf[:, :])
                s0p = pc.tile([1, 1], DT.float32, tag="s0p")
                nc.tensor.matmul(out=s0p[:], lhsT=zc_sb[:], rhs=tot[:, 2:3],
                                 start=True, stop=True)
                s0 = sm.tile([1, 1], DT.float32, tag="s0")
                nc.vector.tensor_scalar(out=s0[:], in0=s0p[:],
                                        scalar1=1.0 / N_NODES, scalar2=c1,
                                        op0=ALU.mult, op1=ALU.add)
                outsb = cp.tile([1, BSH], DT.float32, tag="outsb")
                for c in range(BSH // 512):
                    cs = slice(c * 512, (c + 1) * 512)
                    pov = pc.tile([1, 512], DT.float32, tag="pov")
                    nc.tensor.matmul(out=pov[:], lhsT=vc_sb[:], rhs=hT[:, cs],
                                     start=True, stop=True)
                    nc.vector.tensor_scalar(out=outsb[:, cs], in0=pov[:],
                                            scalar1=s0[:, :], scalar2=None,
                                            op0=ALU.add)
                nc.sync.dma_start(out=out_ap[:], in_=outsb[:])

    nc.compile()
    return nc


_CACHE = {}


def _get_program(T_LO, T_HI, c1, reps=1):
    key = (tuple(T_LO), tuple(T_HI), float(c1), reps)
    if key not in _CACHE:
        _CACHE[key] = _build_program(tuple(T_LO), tuple(T_HI), c1, reps)
    return _CACHE[key]


def _in_maps(st):
    maps = []
    for k in range(CORES):
        m = dict(st["shared"])
        m.update(st["per_core"][k])
        maps.append(m)
    return maps


def kernel(**inputs):
    st = _prep(inputs)
    nc = _get_program(st["T_LO"], st["T_HI"], st["c1"], reps=1)
    res = bass_utils.run_bass_kernel_spmd(
        nc, _in_maps(st), core_ids=list(range(CORES)))
    out = np.concatenate(
        [res.results[k]["out"].reshape(BSH, 1) for k in range(CORES)], axis=0)
    return out.astype(np.float32)


# revision 10
# speedup vs baseline: 1.4402x; 1.0445x over previous
"""Trainium2 Bass kernel: CombinedModel = DNN branch (Linear+BatchNorm+ReLU)
+ GCN branch (2x GCNConv -> mean pool) + linear head, on 8 NeuronCores.

v3: batched InstDMAGatherAnt gathers from a bf16 PAIR table + bf16
one-hot matmuls:
- x2 is repacked host-side as x2pair [25000, 128] bf16 (nodes 2i, 2i+1 per
  256B row) so gather indices (src >> 1) fit int16 and rows satisfy the
  256B-multiple element constraint.
- edges sorted by dest block; within a block split into even-src / odd-src
  streams (sorted by src, padded to 128-edge tiles, pad idx 0 / nrm 0); the
  stream parity selects which half of the gathered pair feeds the matmul.
- dma_gather calls capped at 8 tiles (1024 descriptors = one SWDGE ring)
  round-robin across 4 SWDGE queues (~2.4-6 ns/edge); larger calls overflow
  the ring and can wedge the exec unit.
- one-hot "stiles" and the aggregation matmuls run in bf16 (fp32 PSUM
  accumulate); self-loops use a no-gather diagonal path from SBUF-resident
  x2 rows of the core's own dest range.
Everything else (DNN branch, algebraic GCN layer-2 collapse, folded head,
single tiny AllReduce) is unchanged from the baseline.
"""

import math
import os
import sys

for _p in ("/opt/trn_rl_repo", "/root/.axon_site/_ro/trn_rl_repo"):
    if os.path.isdir(_p) and _p not in sys.path:
        sys.path.append(_p)

import numpy as np
import ml_dtypes

import concourse.bacc as bacc
import concourse.bass as bass
import concourse.mybir as mybir
import concourse.tile as tile
from concourse import bass_utils
from concourse.masks import make_identity

DT = mybir.dt
ALU = mybir.AluOpType
ACTF = mybir.ActivationFunctionType

N_NODES = 50000
N_EDGES = 800000
BATCH = 16384
DNN_IN = 256
F = 64                       # feature width everywhere in the GNN
CORES = 8
NSH = N_NODES // CORES       # 6250 dest nodes per core
BSH = BATCH // CORES         # 2048 batch rows per core
NBLK = (NSH + 127) // 128    # 49 dest blocks per core
NPAIR = N_NODES // 2         # rows in the bf16 pair table
BN_EPS = 1e-5
GBLK = int(os.environ.get("K_GBLK", "1"))   # dest blocks per gather call pair
NQ = int(os.environ.get("K_NQ", "4"))        # SWDGE queues
GBUFS = int(os.environ.get("K_GBUFS", "6"))  # gather pool bufs (each stream)
QRR = int(os.environ.get("K_QRR", "1"))      # 1 = strict round-robin queues
SCRATCH = int(os.environ.get("K_SCRATCH", "32768"))  # SWDGE ring bytes/queue

_PHASES = dict(dnn=True, gcn=True, head=True, gather=True, stile=True, tail=True)


def _cdiv(a, b):
    return (a + b - 1) // b


def _pack_idx16(idx, tiles):
    """idx [n] -> [128, tiles*8] int16: edge e at (e%16, e//16), replicated
    across the 8 groups of 16 partitions (dma_gather wrap layout)."""
    cols = tiles * 8
    a = np.zeros((16, cols), np.int16)
    n = idx.shape[0]
    assert n <= cols * 16
    a[:, : _cdiv(n, 16)] = (
        np.pad(idx.astype(np.int16), (0, _cdiv(n, 16) * 16 - n))
        .reshape(_cdiv(n, 16), 16).T)
    return np.tile(a, (8, 1))


# --------------------------------------------------------------------------
# Host-side preprocessing: graph indices -> per-core packed gather/one-hot
# metadata with a core-uniform tile structure (SPMD requires one program).
# --------------------------------------------------------------------------

def _prep(inputs):
    x1 = np.asarray(inputs["x1"], np.float32)
    x2 = np.ascontiguousarray(np.asarray(inputs["x2"], np.float32))
    ei = np.asarray(inputs["edge_index"])
    row = ei[0].astype(np.int64)
    col = ei[1].astype(np.int64)

    deg = (np.bincount(col, minlength=N_NODES) + 1.0).astype(np.float32)
    dis = (1.0 / np.sqrt(deg)).astype(np.float32)
    norm = dis[row] * dis[col]

    # layer-2 collapse weights: sum_c out2[c] = sum_n wslf[n] * h2[n] + N*b2
    w_r = np.bincount(row, weights=dis[col].astype(np.float64), minlength=N_NODES)
    wslf = (dis * w_r.astype(np.float32) + dis * dis).astype(np.float32)

    order = np.argsort(col, kind="stable")
    srow = row[order]
    scol = col[order]
    snrm = norm[order]

    # per (core, block, stream) segments; stream 0 = even src, 1 = odd src
    segs = [[None] * NBLK for _ in range(CORES)]
    for k in range(CORES):
        base = k * NSH
        s0 = np.searchsorted(scol, base)
        s1 = np.searchsorted(scol, base + NSH)
        krow = srow[s0:s1]
        knrm = snrm[s0:s1]
        rel = scol[s0:s1] - base
        bst = np.searchsorted(rel, np.arange(NBLK) * 128)
        ben = np.append(bst[1:], rel.size)
        for b in range(NBLK):
            sl = slice(bst[b], ben[b])
            r = krow[sl]
            n = knrm[sl]
            c = (rel[sl] - b * 128).astype(np.float32)
            # sort by src: ascending HBM addresses within each stream
            o = np.argsort(r, kind="stable")
            r, n, c = r[o], n[o], c[o]
            ev = (r % 2) == 0
            segs[k][b] = ((r[ev] >> 1, n[ev], c[ev]),
                          (r[~ev] >> 1, n[~ev], c[~ev]))

    T_LO = [max(_cdiv(segs[k][b][0][0].size, 128) for k in range(CORES))
            for b in range(NBLK)]
    T_HI = [max(_cdiv(segs[k][b][1][0].size, 128) for k in range(CORES))
            for b in range(NBLK)]
    # every block also gets one diag (self-loop) tile, built on-device from
    # the SBUF-resident x2 rows of the core's own dest range
    TLOS, THIS = sum(T_LO), sum(T_HI)

    # gather-call groups of GBLK blocks
    groups = [list(range(g, min(g + GBLK, NBLK)))
              for g in range(0, NBLK, GBLK)]

    def pack_core(k):
        # crl/nrm streams in global tile order:
        #   per group: [lo tiles of its blocks][hi tiles of its blocks]
        # then per block one diag tile appended at the very end (NBLK tiles).
        ntile = TLOS + THIS
        nrm = np.zeros((ntile + NBLK) * 128, np.float32)
        crl = np.zeros((ntile + NBLK) * 128, np.float32)
        idx16_parts = []
        off = 0
        for grp in groups:
            for st in range(2):
                gidx = []
                for b in grp:
                    r, n, c = segs[k][b][st]
                    T = (T_LO if st == 0 else T_HI)[b]
                    m = r.size
                    nrm[off:off + m] = n
                    crl[off:off + m] = c
                    gidx.append(np.pad(r.astype(np.int64), (0, T * 128 - m)))
                    off += T * 128
                gidx = np.concatenate(gidx) if gidx else np.zeros(0, np.int64)
                idx16_parts.append(_pack_idx16(gidx, gidx.size // 128))
        # diag tiles: dest d of block b <- x2[k*NSH + b*128 + d] * dis^2
        for b in range(NBLK):
            nvalid = min(128, NSH - b * 128)
            d2 = (dis[k * NSH + b * 128: k * NSH + b * 128 + nvalid]) ** 2
            nrm[off:off + nvalid] = d2
            crl[off:off + nvalid] = np.arange(nvalid, dtype=np.float32)
            off += 128
        ntot = ntile + NBLK
        nrm_t = np.ascontiguousarray(nrm.reshape(ntot, 128).T)
        crl_t = np.ascontiguousarray(crl.reshape(ntot, 128).T)
        idx16 = np.concatenate(idx16_parts, axis=1)
        wk = np.zeros(NBLK * 128, np.float32)
        wk[:NSH] = wslf[k * NSH:(k + 1) * NSH]
        x2blkT = np.zeros((128, NBLK * F), np.float32)
        xk = x2[k * NSH:(k + 1) * NSH]          # [6250, 64]
        for b in range(NBLK):
            nvalid = min(128, NSH - b * 128)
            x2blkT[:nvalid, b * F:(b + 1) * F] = xk[b * 128: b * 128 + nvalid]
        x2blkT = x2blkT.astype(ml_dtypes.bfloat16)
        return dict(
            idx=idx16, nrm=nrm_t, crl=crl_t,
            wslf=np.ascontiguousarray(wk.reshape(NBLK, 128).T),
            x2blk=x2blkT,
            x1t=None,  # filled below
        )

    per_core = []
    x1t_full = np.ascontiguousarray(x1.T)
    for k in range(CORES):
        m = pack_core(k)
        m["x1t"] = np.ascontiguousarray(x1t_full[:, k * BSH:(k + 1) * BSH])
        per_core.append(m)

    # host-folded head weights (no nonlinearity between fc1 and fc2)
    fc1 = np.asarray(inputs["fc1_W"], np.float32)
    fc2 = np.asarray(inputs["fc2_W"], np.float32)
    u = fc1[F:, :] @ fc2                                    # [64, 1]
    v = np.ascontiguousarray(fc1[:F, :] @ fc2)              # [64, 1]
    z = np.ascontiguousarray(np.asarray(inputs["gcn2_W"], np.float32) @ u)
    c1 = float(np.asarray(inputs["fc1_b"], np.float32) @ fc2[:, 0]
               + np.asarray(inputs["fc2_b"], np.float32)[0]
               + np.asarray(inputs["gcn2_b"], np.float32) @ u[:, 0])

    x2pair = np.ascontiguousarray(
        x2.reshape(NPAIR, 2 * F).astype(ml_dtypes.bfloat16))
    shared = dict(
        x2pair=x2pair,
        w1=np.ascontiguousarray(np.asarray(inputs["gcn1_W"], np.float32)),
        b1b=np.ascontiguousarray(
            np.tile(np.asarray(inputs["gcn1_b"], np.float32), (128, 1))),
        dnnw=np.ascontiguousarray(np.asarray(inputs["dnn_W"], np.float32)),
        gma=np.ascontiguousarray(
            np.asarray(inputs["bn_gamma"], np.float32).reshape(F, 1)),
        bta=np.ascontiguousarray(
            np.asarray(inputs["bn_beta"], np.float32).reshape(F, 1)),
        vc=v, zc=z,
        iota=np.ascontiguousarray(
            np.broadcast_to(np.arange(128, dtype=np.float32), (128, 128))
            .astype(ml_dtypes.bfloat16)),
    )
    return dict(T_LO=tuple(T_LO), T_HI=tuple(T_HI), c1=c1,
                per_core=per_core, shared=shared)


# --------------------------------------------------------------------------
# Device program
# --------------------------------------------------------------------------

def _build_program(T_LO, T_HI, c1, reps=1):
    TLOS, THIS = sum(T_LO), sum(T_HI)
    NTILE = TLOS + THIS + NBLK           # + diag tiles
    groups = [list(range(g, min(g + GBLK, NBLK)))
              for g in range(0, NBLK, GBLK)]

    nc = bacc.Bacc("TRN2", target_bir_lowering=False, debug=False,
                   enable_asserts=False, num_devices=CORES,
                   num_swdge_queues=NQ, dynamic_dma_scratch_size=SCRATCH)
    ap = {}

    def inp(name, shape, dt=DT.float32):
        ap[name] = nc.dram_tensor(name, list(shape), dt,
                                  kind="ExternalInput").ap()

    inp("x2pair", (NPAIR, 2 * F), DT.bfloat16)
    inp("x1t", (DNN_IN, BSH))
    inp("idx", (128, NTILE * 8 - NBLK * 8), DT.int16)
    inp("nrm", (128, NTILE))
    inp("crl", (128, NTILE))
    inp("wslf", (128, NBLK))
    inp("x2blk", (128, NBLK * F), DT.bfloat16)
    inp("w1", (F, F))
    inp("b1b", (128, F))
    inp("dnnw", (DNN_IN, F))
    inp("gma", (F, 1))
    inp("bta", (F, 1))
    inp("vc", (F, 1))
    inp("zc", (F, 1))
    inp("iota", (128, 128), DT.bfloat16)
    out_ap = nc.dram_tensor("out", [1, BSH], DT.float32,
                            kind="ExternalOutput").ap()

    # per-group stream offsets (tiles) and idx16 column offsets
    lo_goff, hi_goff, ic_off = [], [], []
    toff = 0
    icol = 0
    for grp in groups:
        tl = sum(T_LO[b] for b in grp)
        th = sum(T_HI[b] for b in grp)
        lo_goff.append(toff)
        hi_goff.append(toff + tl)
        ic_off.append((icol, icol + tl * 8))
        toff += tl + th
        icol += (tl + th) * 8
    assert toff == TLOS + THIS
    diag_t0 = toff                       # first diag tile index

    with tile.TileContext(nc) as tc:
        with tc.tile_pool(name="const", bufs=1) as cp, \
             tc.tile_pool(name="dram", bufs=1, space="DRAM") as dp:
            def load(name, shape, dt=DT.float32, src=None):
                t = cp.tile(list(shape), dt, tag=name)
                nc.sync.dma_start(out=t[:], in_=src if src is not None
                                  else ap[name][:])
                return t

            w1_sb = load("w1", (F, F))
            b1b_sb = load("b1b", (128, F))
            gma_sb = load("gma", (F, 1))
            bta_sb = load("bta", (F, 1))
            vc_sb = load("vc", (F, 1))
            zc_sb = load("zc", (F, 1))
            iota_sb = load("iota", (128, 128), DT.bfloat16)
            wslf_sb = load("wslf", (128, NBLK))
            x2blk_sb = load("x2blk", (128, NBLK * F), DT.bfloat16)
            dnnw0 = load("dnnw0", (128, F), src=ap["dnnw"][0:128, :])
            dnnw1 = load("dnnw1", (128, F), src=ap["dnnw"][128:256, :])
            x1a = load("x1a", (128, BSH), src=ap["x1t"][0:128, :])
            x1b = load("x1b", (128, BSH), src=ap["x1t"][128:256, :])
            idx_sb = load("idx", (128, NTILE * 8 - NBLK * 8), DT.int16)
            nrm_sb = load("nrm", (128, NTILE))
            crl_sb = load("crl", (128, NTILE))

            ident = cp.tile([128, 128], DT.float32, tag="ident")
            make_identity(nc, ident[:])

            hT = cp.tile([F, BSH], DT.float32, tag="hT")
            sqtmp = cp.tile([F, BSH], DT.float32, tag="sqtmp")
            bn_sum = cp.tile([F, 1], DT.float32, tag="bn_sum")
            bn_sq = cp.tile([F, 1], DT.float32, tag="bn_sq")
            p_acc = cp.tile([F, 1], DT.float32, tag="p_acc")

            def phase_ab():
                if _PHASES["dnn"]:
                    _dnn_phase()
                else:
                    nc.vector.memset(hT[:], 0.0)
                    nc.vector.memset(bn_sum[:], 0.0)
                    nc.vector.memset(bn_sq[:], 1.0)
                    nc.vector.memset(sqtmp[:], 0.0)
                if _PHASES["gcn"]:
                    _gcn_phase()
                else:
                    nc.vector.memset(p_acc[:], 0.0)

            def _dnn_phase():
                with tc.tile_pool(name="pd", bufs=1, space="PSUM") as pd:
                    for c in range(BSH // 512):
                        ps = pd.tile([F, 512], DT.float32)
                        cs = slice(c * 512, (c + 1) * 512)
                        nc.tensor.matmul(out=ps[:], lhsT=dnnw0[:],
                                         rhs=x1a[:, cs], start=True, stop=False)
                        nc.tensor.matmul(out=ps[:], lhsT=dnnw1[:],
                                         rhs=x1b[:, cs], start=False, stop=True)
                        nc.vector.tensor_copy(out=hT[:, cs], in_=ps[:])
                nc.vector.reduce_sum(out=bn_sum[:], in_=hT[:],
                                     axis=mybir.AxisListType.X)
                nc.scalar.activation(out=sqtmp[:], in_=hT[:], func=ACTF.Square,
                                     accum_out=bn_sq[:])

            def _gcn_phase():
                nc.vector.memset(p_acc[:], 0.0)
                # every gather call covers <= CHT tiles; CHT*128 descriptors
                # must stay <= the SWDGE ring size (SCRATCH//16) — larger
                # calls can wedge the exec unit
                CHT = int(os.environ.get("K_CHT", "12"))
                assert CHT * 128 <= SCRATCH // 16
                qctr = [0]
                with tc.tile_pool(name="gp", bufs=2 * GBUFS) as gpool, \
                     tc.tile_pool(name="sp", bufs=12) as sp, \
                     tc.tile_pool(name="wp", bufs=4) as wp, \
                     tc.tile_pool(name="pa", bufs=3, space="PSUM") as pa, \
                     tc.tile_pool(name="pt", bufs=1, space="PSUM") as pt, \
                     tc.tile_pool(name="po", bufs=2, space="PSUM") as po, \
                     tc.tile_pool(name="pb", bufs=2, space="PSUM") as pb:

                    def gather_chunks(t0_tile, icol0, ntiles, half):
                        """Gather `ntiles` stream tiles in <=CHT-tile calls
                        of balanced sizes (11 -> 6+5, not 8+3).
                        Returns [(pool_tile, local_j, global_g)] per tile."""
                        outs = []
                        nch = _cdiv(ntiles, CHT)
                        a = 0
                        for ci in range(nch):
                            nt = _cdiv(ntiles - a, nch - ci)
                            gt = gpool.tile([128, CHT, 2 * F], DT.bfloat16,
                                            tag="gt")
                            if _PHASES["gather"]:
                                nc.gpsimd.dma_gather(
                                    gt[:, 0:nt, :], ap["x2pair"][:],
                                    idx_sb[:, icol0 + a * 8:
                                           icol0 + (a + nt) * 8],
                                    nt * 128, nt * 128, 2 * F,
                                    single_packet=False,
                                    queue_num=qctr[0] % NQ)
                                qctr[0] += 1
                            else:
                                nc.vector.memset(
                                    gt[:].rearrange("p t e -> p (t e)"), 0.0)
                            for j in range(nt):
                                outs.append((gt, j, t0_tile + a + j, half))
                            a += nt
                        return outs

                    for gi, grp in enumerate(groups):
                        tl = sum(T_LO[b] for b in grp)
                        ic0, ic1 = ic_off[gi]
                        lo_base = lo_goff[gi]
                        hi_base = hi_goff[gi]
                        lt_off = 0
                        ht_off = 0
                        for b in grp:
                            ntl, nth = T_LO[b], T_HI[b]
                            tiles = (gather_chunks(lo_base + lt_off,
                                                   ic0 + lt_off * 8, ntl, 0)
                                     + gather_chunks(hi_base + ht_off,
                                                     ic0 + tl * 8 + ht_off * 8,
                                                     nth, 1))
                            agg = pa.tile([128, F], DT.float32)
                            ntot = len(tiles) + 1
                            for ti, (gt, j, g, half) in enumerate(tiles):
                                _edge_mm(sp, agg, iota_sb, crl_sb, nrm_sb, g,
                                         gt[:, j, half * F:(half + 1) * F],
                                         ti, ntot)
                            # diag (self-loop) tile from SBUF-resident x2 rows
                            g = diag_t0 + b
                            _edge_mm(sp, agg, iota_sb, crl_sb, nrm_sb, g,
                                     x2blk_sb[:, b * F:(b + 1) * F],
                                     ntot - 1, ntot)
                            if _PHASES["tail"]:
                                _block_tail(wp, pt, po, pb, agg, b)
                            lt_off += ntl
                            ht_off += nth

            def _edge_mm(sp, agg, iota_sb, crl_sb, nrm_sb, g, rhs, ti, ntot):
                if _PHASES["stile"]:
                    stile = sp.tile([128, 128], DT.bfloat16, tag="stile")
                    nc.vector.tensor_scalar(
                        out=stile[:], in0=iota_sb[:],
                        scalar1=crl_sb[:, g:g + 1],
                        scalar2=nrm_sb[:, g:g + 1],
                        op0=ALU.is_equal, op1=ALU.mult)
                    lhsT = stile[:]
                else:
                    lhsT = iota_sb[:]
                nc.tensor.matmul(out=agg[:], lhsT=lhsT, rhs=rhs,
                                 start=(ti == 0), stop=(ti == ntot - 1))

            def _block_tail(wp, pt, po, pb, agg, b):
                aggsb = wp.tile([128, F], DT.float32, tag="aggsb")
                nc.scalar.activation(out=aggsb[:], in_=agg[:], func=ACTF.Copy)
                pst = pt.tile([F, 128], DT.float32)
                nc.tensor.transpose(out=pst[:], in_=aggsb[:],
                                    identity=ident[:])
                aggT = wp.tile([F, 128], DT.float32, tag="aggT")
                nc.scalar.activation(out=aggT[:], in_=pst[:], func=ACTF.Copy)
                o1 = po.tile([128, F], DT.float32)
                nc.tensor.matmul(out=o1[:], lhsT=aggT[:], rhs=w1_sb[:],
                                 start=True, stop=True)
                g1 = wp.tile([128, F], DT.float32, tag="g1")
                nc.vector.tensor_tensor(out=g1[:], in0=o1[:], in1=b1b_sb[:],
                                        op=ALU.add)
                nc.scalar.activation(out=g1[:], in_=g1[:], func=ACTF.Relu)
                pbt = pb.tile([F, 1], DT.float32)
                nc.tensor.matmul(out=pbt[:], lhsT=g1[:],
                                 rhs=wslf_sb[:, b:b + 1], start=True,
                                 stop=True)
                nc.vector.tensor_tensor(out=p_acc[:], in0=p_acc[:],
                                        in1=pbt[:], op=ALU.add)

            if reps == 1:
                phase_ab()
            else:
                with tc.For_i(0, reps, 1):
                    phase_ab()

            # ---------------- cross-core stats + head ----------------
            stats = cp.tile([F, 4], DT.float32, tag="stats")
            nc.vector.tensor_copy(out=stats[:, 0:1], in_=bn_sum[:])
            nc.vector.tensor_copy(out=stats[:, 1:2], in_=bn_sq[:])
            nc.vector.tensor_copy(out=stats[:, 2:3], in_=p_acc[:])
            cc_in = dp.tile([F, 3], DT.float32)
            cc_out = dp.tile([F, 3], DT.float32)
            nc.gpsimd.dma_start(out=cc_in[:], in_=stats[:, 0:3])
            nc.gpsimd.collective_compute(
                "AllReduce", ALU.add,
                replica_groups=[list(range(CORES))],
                ins=[cc_in.opt()], outs=[cc_out.opt()],
            )
            tot = cp.tile([F, 3], DT.float32, tag="tot")
            nc.sync.dma_start(out=tot[:], in_=cc_out[:])

            if not _PHASES["head"]:
                outsb0 = cp.tile([1, BSH], DT.float32, tag="outsb")
                nc.vector.memset(outsb0[:], 0.0)
                nc.sync.dma_start(out=out_ap[:], in_=outsb0[:])
            if _PHASES["head"]:
              with tc.tile_pool(name="pc", bufs=2, space="PSUM") as pc:
                sm = cp
                mu = sm.tile([F, 1], DT.float32, tag="mu")
                nc.vector.tensor_scalar(out=mu[:], in0=tot[:, 0:1],
                                        scalar1=1.0 / BATCH, scalar2=None,
                                        op0=ALU.mult)
                ex2 = sm.tile([F, 1], DT.float32, tag="ex2")
                nc.vector.tensor_scalar(out=ex2[:], in0=tot[:, 1:2],
                                        scalar1=1.0 / BATCH, scalar2=None,
                                        op0=ALU.mult)
                m2 = sm.tile([F, 1], DT.float32, tag="m2")
                nc.vector.tensor_tensor(out=m2[:], in0=mu[:], in1=mu[:],
                                        op=ALU.mult)
                var = sm.tile([F, 1], DT.float32, tag="var")
                nc.vector.tensor_tensor(out=var[:], in0=ex2[:], in1=m2[:],
                                        op=ALU.subtract)
                vp = sm.tile([F, 1], DT.float32, tag="vp")
                nc.vector.tensor_scalar(out=vp[:], in0=var[:],
                                        scalar1=BN_EPS, scalar2=None,
                                        op0=ALU.add)
                sd = sm.tile([F, 1], DT.float32, tag="sd")
                nc.scalar.activation(out=sd[:], in_=vp[:], func=ACTF.Sqrt)
                istd = sm.tile([F, 1], DT.float32, tag="istd")
                nc.vector.reciprocal(out=istd[:], in_=sd[:])
                scl = sm.tile([F, 1], DT.float32, tag="scl")
                nc.vector.tensor_tensor(out=scl[:], in0=istd[:], in1=gma_sb[:],
                                        op=ALU.mult)
                msc = sm.tile([F, 1], DT.float32, tag="msc")
                nc.vector.tensor_tensor(out=msc[:], in0=mu[:], in1=scl[:],
                                        op=ALU.mult)
                shf = sm.tile([F, 1], DT.float32, tag="shf")
                nc.vector.tensor_tensor(out=shf[:], in0=bta_sb[:], in1=msc[:],
                                        op=ALU.subtract)
                nc.scalar.activation(out=hT[:], in_=hT[:], func=ACTF.Relu,
                                     scale=scl[:, :], bias=shf[:, :])
                s0p = pc.tile([1, 1], DT.float32, tag="s0p")
                nc.tensor.matmul(out=s0p[:], lhsT=zc_sb[:], rhs=tot[:, 2:3],
                                 start=True, stop=True)
                s0 = sm.tile([1, 1], DT.float32, tag="s0")
                nc.vector.tensor_scalar(out=s0[:], in0=s0p[:],
                                        scalar1=1.0 / N_NODES, scalar2=c1,
                                        op0=ALU.mult, op1=ALU.add)
                outsb = cp.tile([1, BSH], DT.float32, tag="outsb")
                for c in range(BSH // 512):
                    cs = slice(c * 512, (c + 1) * 512)
                    pov = pc.tile([1, 512], DT.float32, tag="pov")
                    nc.tensor.matmul(out=pov[:], lhsT=vc_sb[:], rhs=hT[:, cs],
                                     start=True, stop=True)
                    nc.vector.tensor_scalar(out=outsb[:, cs], in0=pov[:],
                                            scalar1=s0[:, :], scalar2=None,
                                            op0=ALU.add)
                nc.sync.dma_start(out=out_ap[:], in_=outsb[:])

    nc.compile()
    return nc


_CACHE = {}


def _get_program(T_LO, T_HI, c1, reps=1):
    key = (tuple(T_LO), tuple(T_HI), float(c1), reps)
    if key not in _CACHE:
        _CACHE[key] = _build_program(tuple(T_LO), tuple(T_HI), c1, reps)
    return _CACHE[key]


def _in_maps(st):
    maps = []
    for k in range(CORES):
        m = dict(st["shared"])
        m.update(st["per_core"][k])
        maps.append(m)
    return maps


def kernel(**inputs):
    st = _prep(inputs)
    nc = _get_program(st["T_LO"], st["T_HI"], st["c1"], reps=1)
    res = bass_utils.run_bass_kernel_spmd(
        nc, _in_maps(st), core_ids=list(range(CORES)))
    out = np.concatenate(
        [res.results[k]["out"].reshape(BSH, 1) for k in range(CORES)], axis=0)
    return out.astype(np.float32)
